# revision 34
# baseline (speedup 1.0000x reference)
"""Trainium2 Bass kernel for nn_LinkPredictor (MoE-routed bilinear link scorer).

score[b] = head[b]^T @ W[rel_id[b]] @ tail[b] + sum(b[rel_id[b]])

Strategy (relation sharding / MoE routing on host, dense matmuls on device):
  * Host groups samples by relation (argsort of rel_id), splits each
    relation's samples into slots of <=128, and assigns slots to the 8
    NeuronCores balanced by sample count.  Each core sees S slots; slot j
    has a static per-slot capacity cap_j (max over cores, 32-aligned).
  * Per slot the device computes Q = H_slot @ W[r] via 4 PE matmuls
    (contraction over e in chunks of 128; stationary = transposed heads
    [e_chunk, samples], moving = W[r][e_chunk, :] streamed at N=512 fp16),
    accumulated in one PSUM bank as Q[sample, d].
  * DVE: per slot one affine_mul_reduce fuses Q*tail with the row-sum;
    one final tensor_add applies the relation-bias columns, which ride as
    S fp16 columns at the tail of the heads tensor (no separate bias DMA).
  * Everything streams as fp16 (tails and bias too): per-core traffic is
    W 2.05 MB + heads 0.33 + tails 0.33 = 2.7 MB, vs 3.08 MB with fp32
    tails.  W is read exactly once from HBM across the machine (16 MB
    fp16 total), which is the bandwidth floor for this sharding.
  * Queue plan (trace-tuned): W halves ride the sync HWDGE ring with the
    PE's first gate (slot0 h0) at the ring head -- ring-head completions
    are clean while mid-ring completion semaphores can trail their data
    by 1-4 us (one straggler DMA engine out of 16).  Slots 0-1 (the
    biggest) go on sync; heads-first + tails + slots 2..S-1 go on scalar
    so both rings drain together.  Every DMA is a single linear DRAM run
    with 1-2.5 KB descriptors.
  * The out-store's completion is NOT waited on: the NEFF epilogue drains
    all DMA queues anyway, so the block ends right after the store issues
    and the store's flight overlaps the fixed ~8 us teardown epilogue.
"""

import os
import sys
import math

import numpy as np

for _p in ("/opt/trn_rl_repo",):
    if _p not in sys.path:
        sys.path.append(_p)

import concourse.bass as bass  # noqa: E402
import concourse.mybir as mybir  # noqa: E402
from concourse import bacc  # noqa: E402
from concourse import bass_utils  # noqa: E402

B, D, R = 2048, 512, 32
N_CORES = 8
F32 = mybir.dt.float32
F16 = mybir.dt.float16
NP16 = np.float16

WAIT_OUT = os.environ.get("BASS_WAIT_OUT", "0") == "1"


def _install_ntff_hook():
    """Provide antenv.axon_hooks if the image lacks it, so trace=True /
    BASS_TRACE=1 profiling works under axon (see trn_agent_boot.trn_boot)."""
    try:
        from antenv.axon_hooks import get_axon_ntff_profile_hook  # noqa: F401
        return
    except ImportError:
        pass
    import types
    try:
        import antenv
        from trn_agent_boot.trn_boot import _ntff_profile_via_ctypes
    except Exception:
        return
    mod = types.ModuleType("antenv.axon_hooks")
    _state = {"hook": None}
    try:
        _state["hook"] = _ntff_profile_via_ctypes("/opt/axon/libaxon_pjrt.so")
    except Exception:
        _state["hook"] = None

    def set_axon_ntff_profile_hook(h):
        _state["hook"] = h

    def get_axon_ntff_profile_hook():
        return _state["hook"]

    mod.set_axon_ntff_profile_hook = set_axon_ntff_profile_hook
    mod.get_axon_ntff_profile_hook = get_axon_ntff_profile_hook
    sys.modules["antenv.axon_hooks"] = mod
    antenv.axon_hooks = mod


_install_ntff_hook()

_PROGRAM_CACHE = {}


def _build_program(S, caps):
    """Raw-bacc program for one core: S slots, slot j holds cap_j samples of
    one relation.  caps is a tuple of per-slot capacities (<=128)."""
    caps = list(caps)
    M = sum(caps)
    offs = [0]
    for c_ in caps:
        offs.append(offs[-1] + c_)
    assert S <= 8, "need one PSUM bank per slot"

    nc = bacc.Bacc("TRN2", target_bir_lowering=False, debug=False,
                   num_devices=N_CORES)

    # hb: transposed heads by e-chunk [p, c*M + m] plus S fp16 bias columns.
    hb = nc.dram_tensor("hb", [128, 4 * M + S], F16, kind="ExternalInput")
    # tl: per-sample tails (fp16), slot-contiguous rows.
    tl = nc.dram_tensor("tl", [M, D], F16, kind="ExternalInput")
    # wc: W slots; slot j is rows [j*128, (j+1)*128) = one linear 512 KB
    # run with 4 KB descriptors (the DMA pool runs fastest with big runs).
    wc = nc.dram_tensor("wc", [S * 128, 4 * D], F16, kind="ExternalInput")
    out = nc.dram_tensor("out", [128, S], F32, kind="ExternalOutput")

    import contextlib
    with contextlib.ExitStack() as ctx:
        block = ctx.enter_context(nc.Block())
        # One semaphore per DMA (a sem shared by several DMAs only supports
        # waits at the final total: per-engine completions interleave).
        sem_w = [ctx.enter_context(nc.semaphore(f"sem_w{k}"))
                 for k in range(S)]
        sem_t = [ctx.enter_context(nc.semaphore(f"sem_t{j}"))
                 for j in range(S)]
        sem_hb = ctx.enter_context(nc.semaphore("sem_hb"))
        sem_mm = ctx.enter_context(nc.semaphore("sem_mm"))  # PE -> DVE
        sem_r = ctx.enter_context(nc.semaphore("sem_r"))    # DVE red -> add
        sem_v = ctx.enter_context(nc.semaphore("sem_v"))    # DVE -> out DMA
        sem_o = ctx.enter_context(nc.semaphore("sem_o"))    # out DMA done
        sem_z = ctx.enter_context(nc.semaphore("sem_z"))    # memset done

        hb_t = ctx.enter_context(nc.sbuf_tensor("hbt", [128, 4 * M + S], F16))
        w_t = [ctx.enter_context(
            nc.sbuf_tensor(f"w{j}", [128, 4, D], F16)) for j in range(S)]
        tl_t = [ctx.enter_context(
            nc.sbuf_tensor(f"tl{j}", [caps[j], D], F16)) for j in range(S)]
        prod_t = [ctx.enter_context(
            nc.sbuf_tensor(f"prod{j}", [caps[j], D], F32)) for j in range(S)]
        score_t = ctx.enter_context(nc.sbuf_tensor("score", [128, S], F32))
        final_t = ctx.enter_context(nc.sbuf_tensor("final", [128, S], F32))
        psum_t = [ctx.enter_context(
            nc.psum_tensor(f"P{j}", [128, D], F32)) for j in range(S)]

        def w_slot_dma(eng, j):
            eng.dma_start(
                w_t[j].ap(),
                wc.ap()[j * 128:(j + 1) * 128, :].rearrange(
                    "p (c d) -> p c d", c=4),
            ).then_inc(sem_w[j], 16)

        def tl_dma(eng, j):
            eng.dma_start(
                tl_t[j].ap(), tl.ap()[offs[j]:offs[j + 1], :]
            ).then_inc(sem_t[j], 16)

        half = min(2, S)  # slots on the sync ring (after hb)

        @block.sync
        def _(sync):
            sync.dma_start(hb_t.ap(), hb.ap()).then_inc(sem_hb, 16)
            for j in range(half):
                w_slot_dma(sync, j)
            # output store, gated on the final DVE bias-add; completion is
            # covered by the NEFF epilogue's queue drain (no wait).
            sync.wait_ge(sem_v, 1)
            sync.dma_start(out.ap(), final_t.ap()).then_inc(sem_o, 16)
            if WAIT_OUT:
                sync.wait_ge(sem_o, 16)

        @block.scalar
        def _(scalar):
            nxt = 0  # next tl slot to issue; tl_j must precede reduce j
            for j in range(half, S):
                while nxt < j:
                    tl_dma(scalar, nxt)
                    nxt += 1
                w_slot_dma(scalar, j)
            while nxt < S:
                tl_dma(scalar, nxt)
                nxt += 1

        @block.gpsimd
        def _(gpsimd):
            # zero the score scratch (junk rows beyond cap_j stay finite)
            gpsimd.memset(score_t.ap(), 0.0).then_inc(sem_z, 1)

        @block.tensor
        def _(tensor):
            tensor.wait_ge(sem_hb, 16)
            for j in range(S):
                for c in range(4):
                    if c == 0:
                        tensor.wait_ge(sem_w[j], 16)
                    mm = tensor.matmul(
                        psum_t[j].ap()[:caps[j], :],
                        hb_t.ap()[:, c * M + offs[j]:c * M + offs[j + 1]],
                        w_t[j].ap()[:, c, :],
                        start=(c == 0),
                        stop=(c == 3),
                    )
                mm.then_inc(sem_mm, 1)

        @block.vector
        def _(vector):
            vector.wait_ge(sem_z, 1)
            for j in range(S):
                vector.wait_ge(sem_mm, j + 1)
                vector.wait_ge(sem_t[j], 16)
                if j == 0:
                    vector.wait_ge(sem_hb, 16)
                # score[:,j] = sum_d(Q*tail), fused mul+row-sum in one DVE op
                vector.affine_mul_reduce(
                    out=prod_t[j].ap(),
                    accum_out=score_t.ap()[:caps[j], j:j + 1],
                    in0=psum_t[j].ap()[:caps[j], :],
                    in1=tl_t[j].ap(),
                    scale=1.0, bias=0.0,
                ).then_inc(sem_r, 1)
            # relation-bias columns live at the tail of hb (fp16)
            vector.wait_ge(sem_r, S)
            vector.tensor_add(
                final_t.ap(), score_t.ap(),
                hb_t.ap()[:, 4 * M:4 * M + S]).then_inc(sem_v, 1)

    nc.compile()
    return nc


def _route(rel):
    """Group samples by relation into slots of <=128; balance across cores.

    Returns (S, caps, core_slots): core_slots[c] is a list of exactly S
    (relation, sample_indices) pairs, sorted by size desc; caps[j] is the
    static capacity of slot j (max count over cores, 32-aligned)."""
    counts = np.bincount(rel, minlength=R)
    order = np.argsort(rel, kind="stable")
    slots = []
    off = 0
    for r in range(R):
        n = int(counts[r])
        idx = order[off:off + n]
        off += n
        for c0 in range(0, n, 128):
            slots.append((r, idx[c0:c0 + 128]))
    S = max(1, math.ceil(len(slots) / N_CORES))
    # Greedy balance: biggest slots first onto least-loaded core with room.
    slots.sort(key=lambda s: -len(s[1]))
    core_slots = [[] for _ in range(N_CORES)]
    loads = [0] * N_CORES
    for r, idx in slots:
        cands = [c for c in range(N_CORES) if len(core_slots[c]) < S]
        c = min(cands, key=lambda c: loads[c])
        core_slots[c].append((r, idx))
        loads[c] += len(idx)
    empty = np.zeros(0, dtype=np.int64)
    for c in range(N_CORES):
        core_slots[c].sort(key=lambda s: -len(s[1]))
        while len(core_slots[c]) < S:
            core_slots[c].append((0, empty))
    caps = tuple(
        min(128, max(32, 32 * math.ceil(
            max(len(core_slots[c][j][1]) for c in range(N_CORES)) / 32)))
        for j in range(S))
    return S, caps, core_slots


def _marshal(head_emb, tail_emb, rel, W, b):
    """Route + build per-core input maps (device-ready layouts)."""
    S, caps, core_slots = _route(rel)
    offs = np.concatenate([[0], np.cumsum(caps)]).astype(int)
    M = int(offs[-1])
    bsum = b.astype(np.float64).sum(axis=1).astype(NP16)

    in_maps = []
    for c in range(N_CORES):
        hbm = np.zeros((128, 4 * M + S), dtype=NP16)
        tlm = np.zeros((M, D), dtype=NP16)
        wcm = np.empty((S * 128, 4 * D), dtype=NP16)
        for j, (r, idx) in enumerate(core_slots[c]):
            n = len(idx)
            o = offs[j]
            if n:
                # hb[p, cc*M + o+k] = head_emb[idx_k, cc*128+p]
                ht4 = head_emb[idx].T.reshape(4, 128, n)
                for cc in range(4):
                    hbm[:, cc * M + o:cc * M + o + n] = ht4[cc]
                tlm[o:o + n, :] = tail_emb[idx]
            hbm[:, 4 * M + j] = bsum[r]
            # wc row block j = all 4 e-chunks for each partition, so the
            # slot DMA is one linear 512 KB run with 4 KB descriptors
            wcm[j * 128:(j + 1) * 128, :] = (
                W[r].reshape(4, 128, D).transpose(1, 0, 2).reshape(128, 4 * D))
        in_maps.append({"hb": hbm, "tl": tlm, "wc": wcm})
    return S, caps, core_slots, in_maps


def kernel(head_emb, tail_emb, rel_id, W, b, **_unused):
    head_emb = np.ascontiguousarray(np.asarray(head_emb, dtype=np.float32))
    tail_emb = np.ascontiguousarray(np.asarray(tail_emb, dtype=np.float32))
    W = np.ascontiguousarray(np.asarray(W, dtype=np.float32))
    b = np.ascontiguousarray(np.asarray(b, dtype=np.float32))
    rel = np.asarray(rel_id).astype(np.int64)

    S, caps, core_slots, in_maps = _marshal(head_emb, tail_emb, rel, W, b)

    key = (S, caps)
    if key not in _PROGRAM_CACHE:
        _PROGRAM_CACHE[key] = _build_program(S, caps)
    nc = _PROGRAM_CACHE[key]

    res = bass_utils.run_bass_kernel_spmd(nc, in_maps,
                                          core_ids=list(range(N_CORES)))

    scores = np.zeros(B, dtype=np.float32)
    for c in range(N_CORES):
        o = res.results[c]["out"]
        for j, (r, idx) in enumerate(core_slots[c]):
            n = len(idx)
            if n:
                scores[idx] = o[:n, j]
    return scores


# revision 35
# speedup vs baseline: 1.0351x; 1.0351x over previous
"""Trainium2 Bass kernel for nn_LinkPredictor (MoE-routed bilinear link scorer).

score[b] = head[b]^T @ W[rel_id[b]] @ tail[b] + sum(b[rel_id[b]])

Strategy (relation sharding / MoE routing on host, dense matmuls on device):
  * Host groups samples by relation (argsort of rel_id), splits each
    relation's samples into slots of <=128, and assigns slots to the 8
    NeuronCores balanced by sample count.  Each core sees S slots; slot j
    has a static per-slot capacity cap_j (max over cores, 32-aligned).
  * Per slot the device computes Q = H_slot @ W[r] via 4 PE matmuls
    (contraction over e in chunks of 128; stationary = transposed heads
    [e_chunk, samples], moving = W[r][e_chunk, :] streamed at N=512 fp16),
    accumulated in one PSUM bank as Q[sample, d].
  * DVE: per slot one affine_mul_reduce fuses Q*tail with the row-sum;
    one final tensor_add applies the relation-bias columns, which ride as
    S fp16 columns at the tail of the heads tensor (no separate bias DMA).
  * Everything streams as fp16 (tails and bias too): per-core traffic is
    W 2.05 MB + heads 0.33 + tails 0.33 = 2.7 MB, vs 3.08 MB with fp32
    tails.  W is read exactly once from HBM across the machine (16 MB
    fp16 total), which is the bandwidth floor for this sharding.
  * Queue plan (trace-tuned): W halves ride the sync HWDGE ring with the
    PE's first gate (slot0 h0) at the ring head -- ring-head completions
    are clean while mid-ring completion semaphores can trail their data
    by 1-4 us (one straggler DMA engine out of 16).  Slots 0-1 (the
    biggest) go on sync; heads-first + tails + slots 2..S-1 go on scalar
    so both rings drain together.  Every DMA is a single linear DRAM run
    with 1-2.5 KB descriptors.
  * The out-store's completion is NOT waited on: the NEFF epilogue drains
    all DMA queues anyway, so the block ends right after the store issues
    and the store's flight overlaps the fixed ~8 us teardown epilogue.
"""

import os
import sys
import math

import numpy as np

for _p in ("/opt/trn_rl_repo",):
    if _p not in sys.path:
        sys.path.append(_p)

import concourse.bass as bass  # noqa: E402
import concourse.mybir as mybir  # noqa: E402
from concourse import bacc  # noqa: E402
from concourse import bass_utils  # noqa: E402

B, D, R = 2048, 512, 32
N_CORES = 8
F32 = mybir.dt.float32
F16 = mybir.dt.float16
NP16 = np.float16

WAIT_OUT = os.environ.get("BASS_WAIT_OUT", "0") == "1"


def _install_ntff_hook():
    """Provide antenv.axon_hooks if the image lacks it, so trace=True /
    BASS_TRACE=1 profiling works under axon (see trn_agent_boot.trn_boot)."""
    try:
        from antenv.axon_hooks import get_axon_ntff_profile_hook  # noqa: F401
        return
    except ImportError:
        pass
    import types
    try:
        import antenv
        from trn_agent_boot.trn_boot import _ntff_profile_via_ctypes
    except Exception:
        return
    mod = types.ModuleType("antenv.axon_hooks")
    _state = {"hook": None}
    try:
        _state["hook"] = _ntff_profile_via_ctypes("/opt/axon/libaxon_pjrt.so")
    except Exception:
        _state["hook"] = None

    def set_axon_ntff_profile_hook(h):
        _state["hook"] = h

    def get_axon_ntff_profile_hook():
        return _state["hook"]

    mod.set_axon_ntff_profile_hook = set_axon_ntff_profile_hook
    mod.get_axon_ntff_profile_hook = get_axon_ntff_profile_hook
    sys.modules["antenv.axon_hooks"] = mod
    antenv.axon_hooks = mod


_install_ntff_hook()

_PROGRAM_CACHE = {}


def _build_program(S, caps):
    """Raw-bacc program for one core: S slots, slot j holds cap_j samples of
    one relation.  caps is a tuple of per-slot capacities (<=128)."""
    caps = list(caps)
    M = sum(caps)
    offs = [0]
    for c_ in caps:
        offs.append(offs[-1] + c_)
    assert S <= 8, "need one PSUM bank per slot"

    nc = bacc.Bacc("TRN2", target_bir_lowering=False, debug=False,
                   num_devices=N_CORES)

    # hb: transposed heads by e-chunk [p, c*M + m] plus S fp16 bias columns.
    hb = nc.dram_tensor("hb", [128, 4 * M + S], F16, kind="ExternalInput")
    # tl: per-sample tails (fp16), slot-contiguous rows.
    tl = nc.dram_tensor("tl", [M, D], F16, kind="ExternalInput")
    # wc: W slots; slot j is rows [j*128, (j+1)*128) = one linear 512 KB
    # run with 4 KB descriptors (the DMA pool runs fastest with big runs).
    wc = nc.dram_tensor("wc", [S * 128, 4 * D], F16, kind="ExternalInput")
    out = nc.dram_tensor("out", [128, S], F32, kind="ExternalOutput")

    import contextlib
    with contextlib.ExitStack() as ctx:
        block = ctx.enter_context(nc.Block())
        # One semaphore per DMA (a sem shared by several DMAs only supports
        # waits at the final total: per-engine completions interleave).
        sem_w = [ctx.enter_context(nc.semaphore(f"sem_w{k}"))
                 for k in range(S)]
        sem_t = [ctx.enter_context(nc.semaphore(f"sem_t{j}"))
                 for j in range(S)]
        sem_hb = ctx.enter_context(nc.semaphore("sem_hb"))
        sem_mm = ctx.enter_context(nc.semaphore("sem_mm"))  # PE -> DVE
        sem_r = ctx.enter_context(nc.semaphore("sem_r"))    # DVE red -> add
        sem_v = ctx.enter_context(nc.semaphore("sem_v"))    # DVE -> out DMA
        sem_o = ctx.enter_context(nc.semaphore("sem_o"))    # out DMA done
        sem_z = ctx.enter_context(nc.semaphore("sem_z"))    # memset done

        hb_t = ctx.enter_context(nc.sbuf_tensor("hbt", [128, 4 * M + S], F16))
        w_t = [ctx.enter_context(
            nc.sbuf_tensor(f"w{j}", [128, 4, D], F16)) for j in range(S)]
        tl_t = [ctx.enter_context(
            nc.sbuf_tensor(f"tl{j}", [caps[j], D], F16)) for j in range(S)]
        prod_t = [ctx.enter_context(
            nc.sbuf_tensor(f"prod{j}", [caps[j], D], F32)) for j in range(S)]
        score_t = ctx.enter_context(nc.sbuf_tensor("score", [128, S], F32))
        final_t = ctx.enter_context(nc.sbuf_tensor("final", [128, S], F32))
        psum_t = [ctx.enter_context(
            nc.psum_tensor(f"P{j}", [128, D], F32)) for j in range(S)]

        def w_slot_dma(eng, j):
            eng.dma_start(
                w_t[j].ap(),
                wc.ap()[j * 128:(j + 1) * 128, :].rearrange(
                    "p (c d) -> p c d", c=4),
            ).then_inc(sem_w[j], 16)

        def tl_dma(eng, j):
            eng.dma_start(
                tl_t[j].ap(), tl.ap()[offs[j]:offs[j + 1], :]
            ).then_inc(sem_t[j], 16)

        # Slot 0 (+ hb) on sync; odd slots on scalar, even slots on sync.
        # PE consumes slots in order, so its feed alternates rings: slot1's
        # W streams on scalar while the PE computes slot0 from sync, which
        # removes the inter-slot stall (and lets the PE clock ramp).

        @block.sync
        def _(sync):
            sync.dma_start(hb_t.ap(), hb.ap()).then_inc(sem_hb, 16)
            w_slot_dma(sync, 0)
            for j in range(2, S, 2):
                w_slot_dma(sync, j)
            # output store, gated on the final DVE bias-add; completion is
            # covered by the NEFF epilogue's queue drain (no wait).
            sync.wait_ge(sem_v, 1)
            sync.dma_start(out.ap(), final_t.ap()).then_inc(sem_o, 16)
            if WAIT_OUT:
                sync.wait_ge(sem_o, 16)

        @block.scalar
        def _(scalar):
            nxt = 0  # next tl slot to issue; tl_j must precede reduce j
            for j in range(1, S, 2):
                while nxt < j:
                    tl_dma(scalar, nxt)
                    nxt += 1
                w_slot_dma(scalar, j)
            while nxt < S:
                tl_dma(scalar, nxt)
                nxt += 1

        @block.gpsimd
        def _(gpsimd):
            # zero the score scratch (junk rows beyond cap_j stay finite)
            gpsimd.memset(score_t.ap(), 0.0).then_inc(sem_z, 1)

        @block.tensor
        def _(tensor):
            tensor.wait_ge(sem_hb, 16)
            for j in range(S):
                for c in range(4):
                    if c == 0:
                        tensor.wait_ge(sem_w[j], 16)
                    mm = tensor.matmul(
                        psum_t[j].ap()[:caps[j], :],
                        hb_t.ap()[:, c * M + offs[j]:c * M + offs[j + 1]],
                        w_t[j].ap()[:, c, :],
                        start=(c == 0),
                        stop=(c == 3),
                    )
                mm.then_inc(sem_mm, 1)

        @block.vector
        def _(vector):
            vector.wait_ge(sem_z, 1)
            for j in range(S):
                vector.wait_ge(sem_mm, j + 1)
                vector.wait_ge(sem_t[j], 16)
                if j == 0:
                    vector.wait_ge(sem_hb, 16)
                # score[:,j] = sum_d(Q*tail), fused mul+row-sum in one DVE op
                vector.affine_mul_reduce(
                    out=prod_t[j].ap(),
                    accum_out=score_t.ap()[:caps[j], j:j + 1],
                    in0=psum_t[j].ap()[:caps[j], :],
                    in1=tl_t[j].ap(),
                    scale=1.0, bias=0.0,
                ).then_inc(sem_r, 1)
            # relation-bias columns live at the tail of hb (fp16)
            vector.wait_ge(sem_r, S)
            vector.tensor_add(
                final_t.ap(), score_t.ap(),
                hb_t.ap()[:, 4 * M:4 * M + S]).then_inc(sem_v, 1)

    nc.compile()
    return nc


def _route(rel):
    """Group samples by relation into slots of <=128; balance across cores.

    Returns (S, caps, core_slots): core_slots[c] is a list of exactly S
    (relation, sample_indices) pairs, sorted by size desc; caps[j] is the
    static capacity of slot j (max count over cores, 32-aligned)."""
    counts = np.bincount(rel, minlength=R)
    order = np.argsort(rel, kind="stable")
    slots = []
    off = 0
    for r in range(R):
        n = int(counts[r])
        idx = order[off:off + n]
        off += n
        for c0 in range(0, n, 128):
            slots.append((r, idx[c0:c0 + 128]))
    S = max(1, math.ceil(len(slots) / N_CORES))
    # Greedy balance: biggest slots first onto least-loaded core with room.
    slots.sort(key=lambda s: -len(s[1]))
    core_slots = [[] for _ in range(N_CORES)]
    loads = [0] * N_CORES
    for r, idx in slots:
        cands = [c for c in range(N_CORES) if len(core_slots[c]) < S]
        c = min(cands, key=lambda c: loads[c])
        core_slots[c].append((r, idx))
        loads[c] += len(idx)
    empty = np.zeros(0, dtype=np.int64)
    for c in range(N_CORES):
        core_slots[c].sort(key=lambda s: -len(s[1]))
        while len(core_slots[c]) < S:
            core_slots[c].append((0, empty))
    caps = tuple(
        min(128, max(32, 32 * math.ceil(
            max(len(core_slots[c][j][1]) for c in range(N_CORES)) / 32)))
        for j in range(S))
    return S, caps, core_slots


def _marshal(head_emb, tail_emb, rel, W, b):
    """Route + build per-core input maps (device-ready layouts)."""
    S, caps, core_slots = _route(rel)
    offs = np.concatenate([[0], np.cumsum(caps)]).astype(int)
    M = int(offs[-1])
    bsum = b.astype(np.float64).sum(axis=1).astype(NP16)

    in_maps = []
    for c in range(N_CORES):
        hbm = np.zeros((128, 4 * M + S), dtype=NP16)
        tlm = np.zeros((M, D), dtype=NP16)
        wcm = np.empty((S * 128, 4 * D), dtype=NP16)
        for j, (r, idx) in enumerate(core_slots[c]):
            n = len(idx)
            o = offs[j]
            if n:
                # hb[p, cc*M + o+k] = head_emb[idx_k, cc*128+p]
                ht4 = head_emb[idx].T.reshape(4, 128, n)
                for cc in range(4):
                    hbm[:, cc * M + o:cc * M + o + n] = ht4[cc]
                tlm[o:o + n, :] = tail_emb[idx]
            hbm[:, 4 * M + j] = bsum[r]
            # wc row block j = all 4 e-chunks for each partition, so the
            # slot DMA is one linear 512 KB run with 4 KB descriptors
            wcm[j * 128:(j + 1) * 128, :] = (
                W[r].reshape(4, 128, D).transpose(1, 0, 2).reshape(128, 4 * D))
        in_maps.append({"hb": hbm, "tl": tlm, "wc": wcm})
    return S, caps, core_slots, in_maps


def kernel(head_emb, tail_emb, rel_id, W, b, **_unused):
    head_emb = np.ascontiguousarray(np.asarray(head_emb, dtype=np.float32))
    tail_emb = np.ascontiguousarray(np.asarray(tail_emb, dtype=np.float32))
    W = np.ascontiguousarray(np.asarray(W, dtype=np.float32))
    b = np.ascontiguousarray(np.asarray(b, dtype=np.float32))
    rel = np.asarray(rel_id).astype(np.int64)

    S, caps, core_slots, in_maps = _marshal(head_emb, tail_emb, rel, W, b)

    key = (S, caps)
    if key not in _PROGRAM_CACHE:
        _PROGRAM_CACHE[key] = _build_program(S, caps)
    nc = _PROGRAM_CACHE[key]

    res = bass_utils.run_bass_kernel_spmd(nc, in_maps,
                                          core_ids=list(range(N_CORES)))

    scores = np.zeros(B, dtype=np.float32)
    for c in range(N_CORES):
        o = res.results[c]["out"]
        for j, (r, idx) in enumerate(core_slots[c]):
            n = len(idx)
            if n:
                scores[idx] = o[:n, j]
    return scores
